# revision 6
# baseline (speedup 1.0000x reference)
"""Trainium2 8-core kernel for nn_CAT_81269371175150 (GNN message passing).

Math (see reference):
  gcn(x)   = selu(A_gn @ (x @ W1^T))            for features and aug_features
  S        = softmax_K(gcn1 @ Wt^T)
  loss     = spectral(S, A) + cluster(S) + 0.5 * con(gcn1, gcn2)

Strategy (v3):
  * Nodes sharded row-wise across 8 cores; edge list bucketed by
    (dest block, src chunk) with 4 src chunks, sorted, padded to the
    per-(b,c) max chunk count over cores so all cores run one SPMD program.
  * h = [h1|h2] rows stored fp8e4 (512B/row).  Phase A computes h per src
    chunk and fires that chunk's AllGather immediately -> the 4 AllGathers
    pipeline with phase A's tail and phase B's head.
  * Phase B: per (dest block, src chunk) dma_gather of fp8 rows on round-
    robin SWDGE queues + one fp8 matmul per 128-edge chunk (one-hot lhsT
    with gn folded in) accumulating [A@h1 | A@h2] in PSUM.
  * selu on ScalarE; S = softmax_K written out; log-softmax stats computed
    per node-range (4 ranges) to hide the tail; con-loss partials on-chip.
  * Host finishes tiny reductions (trace(S^T A S), nl, cluster sizes,
    cross-core/range log-softmax merge, final scalar).
"""

import math
import numpy as np
import ml_dtypes

import concourse.bacc as bacc
import concourse.mybir as mybir
import concourse.tile as tile
from concourse import bass_utils
from concourse.masks import make_identity

P = 128
NC = 8
R = 4                     # log-softmax stat ranges

FULL = dict(N=50000, F=500, D=256, K=16)

SELU_L = 1.0507009873554805
SELU_A = 1.6732632423543772
SELU_LA = SELU_L * SELU_A
LN_SELU_LA = math.log(SELU_LA)

CLUSTER_REG = 1.0
CON_REG = 0.5

bf16 = mybir.dt.bfloat16
fp8 = mybir.dt.float8e4
f32 = mybir.dt.float32
i16 = mybir.dt.int16
np_fp8 = ml_dtypes.float8_e4m3

# src chunks in blocks (sum == NB); first small so AllGather 0 fires early
CHUNK_BLOCKS = (7, 14, 14, 14)


def cdiv(a, b):
    return -(-a // b)


# --------------------------------------------------------------------------
# host-side preprocessing
# --------------------------------------------------------------------------

def prep(features, aug_features, graph_row, graph_col, gn_vals, W1, Wt, cfg):
    N, F, D, K = cfg["N"], cfg["F"], cfg["D"], cfg["K"]
    NSH = N // NC
    NB = cdiv(NSH, P)
    NCK = len(CHUNK_BLOCKS)
    cblk = np.asarray(CHUNK_BLOCKS)
    assert cblk.sum() == NB
    cb0 = np.concatenate([[0], np.cumsum(cblk)[:-1]])      # chunk start block
    crow0 = cb0 * P                                        # chunk start row
    crows = np.minimum((cb0 + cblk) * P, NSH) - crow0      # chunk rows

    row = np.asarray(graph_row).astype(np.int64)
    col = np.asarray(graph_col).astype(np.int64)
    gn = np.asarray(gn_vals).astype(np.float64)

    core = row // NSH
    per_core = []
    cnts = np.zeros((NC, NB, NCK), dtype=np.int64)
    for c in range(NC):
        m = core == c
        r = row[m] - c * NSH
        cl = col[m]
        g = gn[m]
        b = r // P
        sr = cl % NSH
        sc = np.searchsorted(crow0, sr, side="right") - 1
        loc = (cl // NSH) * crows[sc] + (sr - crow0[sc])
        order = np.lexsort((cl, sc, b))
        r, cl, g, b, sc, loc = (x[order] for x in (r, cl, g, b, sc, loc))
        key = b * NCK + sc
        cnt = np.bincount(key, minlength=NB * NCK).reshape(NB, NCK)
        cnts[c] = cnt
        per_core.append((r, g, b, sc, loc, key))

    CBC = np.ceil(cnts.max(axis=0) / P).astype(np.int64)   # [NB, NCK]
    nch_b = CBC.sum(axis=1)                                # chunks per block
    NCHT = int(nch_b.sum())
    ohbase = np.concatenate([[0], np.cumsum(nch_b)[:-1]])
    ohsub = np.cumsum(CBC, axis=1) - CBC                   # col offset of (b,c)
    strm_base = np.zeros((NCK, NB), dtype=np.int64)
    for q in range(NCK):
        strm_base[q] = np.concatenate([[0], np.cumsum(CBC[:, q])[:-1]])
    Lc = [int(CBC[:, q].sum()) * P for q in range(NCK)]

    X = np.asarray(features)[0]
    Xa = np.asarray(aug_features)[0]
    XT = np.ascontiguousarray(X.T).astype(ml_dtypes.bfloat16)    # [F, N]
    XTa = np.ascontiguousarray(Xa.T).astype(ml_dtypes.bfloat16)
    W1T = np.ascontiguousarray(np.asarray(W1).T).astype(ml_dtypes.bfloat16)
    WtT = np.ascontiguousarray(np.asarray(Wt).T).astype(ml_dtypes.bfloat16)

    def wrap_idx(a):
        # [L] -> [128, L/16]: element i at [i%16, i//16], replicated x8
        L = a.shape[0]
        w = a.reshape(L // 16, 16).T
        return np.ascontiguousarray(np.tile(w, (8, 1)))

    in_maps = []
    for c in range(NC):
        r, g, b, sc, loc, key = per_core[c]
        cnt = cnts[c]
        run_start = np.zeros(NB * NCK, dtype=np.int64)
        flat = cnt.reshape(-1)
        run_start[1:] = np.cumsum(flat)[:-1]
        rank = np.arange(len(r)) - run_start[key]
        lane = rank % P
        j = rank // P

        idx_streams = []
        for q in range(NCK):
            arr = np.zeros(Lc[q], dtype=np.int16)
            m = sc == q
            off = (strm_base[q][b[m]] + j[m]) * P + lane[m]
            arr[off] = loc[m].astype(np.int16)
            idx_streams.append(wrap_idx(arr))

        oh = np.zeros((P, NCHT, P), dtype=np_fp8)
        ohcol = ohbase[b] + ohsub[b, sc] + j
        dest = r - b * P
        oh[lane, ohcol, dest] = g.astype(np_fp8)

        in_maps.append({
            "xt": np.ascontiguousarray(XT[:, c * NSH:(c + 1) * NSH]),
            "xta": np.ascontiguousarray(XTa[:, c * NSH:(c + 1) * NSH]),
            "w1t": W1T,
            "wtt": WtT,
            "oh": oh,
            **{f"idx{q}": idx_streams[q] for q in range(NCK)},
        })

    meta = dict(
        N=N, F=F, D=D, K=K, NSH=NSH, NB=NB, DT=D // P, FT=cdiv(F, P),
        NCK=NCK, cblk=tuple(cblk.tolist()), cb0=tuple(cb0.tolist()),
        crow0=tuple(crow0.tolist()), crows=tuple(crows.tolist()),
        CBC=tuple(map(tuple, CBC.tolist())), NCHT=NCHT,
        ohbase=tuple(ohbase.tolist()),
        ohsub=tuple(map(tuple, ohsub.tolist())),
        strm_base=tuple(map(tuple, strm_base.tolist())),
        Lc=tuple(Lc),
    )
    return in_maps, meta


# --------------------------------------------------------------------------
# device program
# --------------------------------------------------------------------------

def build(meta, debug=False):
    N, F, D, K = meta["N"], meta["F"], meta["D"], meta["K"]
    NSH, NB, DT, FT = meta["NSH"], meta["NB"], meta["DT"], meta["FT"]
    NCK = meta["NCK"]
    cblk, cb0 = meta["cblk"], meta["cb0"]
    crow0, crows = meta["crow0"], meta["crows"]
    CBC = meta["CBC"]
    NCHT = meta["NCHT"]
    ohbase, ohsub = meta["ohbase"], meta["ohsub"]
    strm_base = meta["strm_base"]
    Lc = meta["Lc"]
    W2 = 2 * D
    VLEN = 2 * D

    # log-softmax ranges: block boundaries
    rblk = [0, 13, 26, 39, NB]
    assert len(rblk) == R + 1

    nc = bacc.Bacc("TRN2", target_bir_lowering=False, debug=debug,
                   num_devices=NC, num_swdge_queues=4)

    xt = nc.dram_tensor("xt", [F, NSH], bf16, kind="ExternalInput")
    xta = nc.dram_tensor("xta", [F, NSH], bf16, kind="ExternalInput")
    w1t = nc.dram_tensor("w1t", [F, D], bf16, kind="ExternalInput")
    wtt = nc.dram_tensor("wtt", [D, K], bf16, kind="ExternalInput")
    oh = nc.dram_tensor("oh", [P, NCHT, P], fp8, kind="ExternalInput")
    idx_d = [nc.dram_tensor(f"idx{q}", [P, Lc[q] // 16], i16,
                            kind="ExternalInput")
             for q in range(NCK)]

    stats_p_d = nc.dram_tensor("stats_p", [P, R * 2 * DT], f32,
                               kind="ExternalOutput")
    stats_v_d = nc.dram_tensor("stats_v", [1, VLEN], f32,
                               kind="ExternalOutput")
    s_out_d = nc.dram_tensor("s_out", [NB * P, K], f32, kind="ExternalOutput")

    max_nch_b = max(sum(CBC[b]) for b in range(NB))
    max_grp = max(CBC[b][q] for b in range(NB) for q in range(NCK))
    crmax = max(crows)

    with tile.TileContext(nc) as tc:
        with (
            tc.tile_pool(name="gbuf", bufs=12) as gbuf,
            tc.tile_pool(name="ohp", bufs=4) as ohp,
            tc.tile_pool(name="persist", bufs=1) as persist,
            tc.tile_pool(name="stage", bufs=4) as stagep,
            tc.tile_pool(name="expp", bufs=2) as expp,
            tc.tile_pool(name="tmp", bufs=4) as tmpp,
            tc.tile_pool(name="small", bufs=4) as smallp,
            tc.tile_pool(name="svp", bufs=1) as svp,
            tc.tile_pool(name="pa", bufs=2, space="PSUM") as pa,
            tc.tile_pool(name="pb", bufs=4, space="PSUM") as pb,
            tc.tile_pool(name="pv", bufs=1, space="PSUM") as pvp,
            tc.tile_pool(name="dram", bufs=1, space="DRAM") as dramp,
        ):
            # ---- constants / resident tensors
            ident = persist.tile([P, P], f32)
            make_identity(nc, ident[:])
            identb = persist.tile([P, P], bf16, tag="identb")
            nc.vector.tensor_copy(identb[:], ident[:])
            w1t_t = persist.tile([P, FT, D], bf16)
            for t in range(FT):
                fr = min(P, F - t * P)
                nc.sync.dma_start(w1t_t[:fr, t, :], w1t[t * P:t * P + fr, :])
            wtt_t = persist.tile([P, DT, K], bf16)
            for t in range(DT):
                nc.sync.dma_start(wtt_t[:, t, :], wtt[t * P:(t + 1) * P, :])
            idx_t = []
            for q in range(NCK):
                it = persist.tile([P, Lc[q] // 16], i16, tag=f"idx{q}")
                nc.sync.dma_start(it[:], idx_d[q][:])
                idx_t.append(it)

            ln_la = persist.tile([P, 1], f32, tag="lnla")
            nc.vector.memset(ln_la[:], LN_SELU_LA)
            la_c = persist.tile([P, 1], f32, tag="lac")
            nc.vector.memset(la_c[:], SELU_LA)
            ones = persist.tile([P, 1], f32, tag="ones")
            nc.vector.memset(ones[:], 1.0)

            gcn1T = persist.tile([P, DT, NB * P], bf16, tag="gcn1T")
            accs = persist.tile([P, VLEN], f32, tag="accs")
            nc.vector.memset(accs[:], 0.0)
            stats_p = persist.tile([P, R * 2 * DT], f32, tag="statsp")

            cc_in = [dramp.tile([crows[q], W2], fp8, name=f"cc_in{q}")
                     for q in range(NCK)]
            cc_out = [dramp.tile([NC * crows[q], W2], fp8,
                                 addr_space="Shared", name=f"cc_out{q}")
                      for q in range(NCK)]

            # ============== phase A + AllGathers (chunk-pipelined) =========
            with tc.tile_pool(name="xp", bufs=4) as xp:
                for q in range(NCK):
                    xts = []
                    for which, src in enumerate((xt, xta)):
                        xx = xp.tile([P, FT, crmax], bf16, tag="xp",
                                     name=f"x_{which}_{q}")
                        for t in range(FT):
                            fr = min(P, F - t * P)
                            nc.sync.dma_start(
                                xx[:fr, t, 0:crows[q]],
                                src[t * P:t * P + fr,
                                    crow0[q]:crow0[q] + crows[q]])
                        xts.append(xx)
                    for b in range(cb0[q], cb0[q] + cblk[q]):
                        boff = b * P - crow0[q]
                        rows = min(P, NSH - b * P)
                        st = stagep.tile([P, W2], fp8, tag="stage")
                        for which in range(2):
                            pt = pb.tile([P, D], f32, space="PSUM", tag="pb")
                            for t in range(FT):
                                fr = min(P, F - t * P)
                                nc.tensor.matmul(
                                    pt[:rows, :],
                                    lhsT=xts[which][:fr, t, boff:boff + rows],
                                    rhs=w1t_t[:fr, t, :],
                                    start=(t == 0), stop=(t == FT - 1),
                                )
                            nc.vector.tensor_copy(
                                st[:rows, which * D:(which + 1) * D],
                                pt[:rows, :])
                        nc.sync.dma_start(cc_in[q][boff:boff + rows, :],
                                          st[:rows, :])
                    nc.gpsimd.collective_compute(
                        "AllGather", mybir.AluOpType.bypass,
                        replica_groups=[list(range(NC))],
                        ins=[cc_in[q][:]], outs=[cc_out[q][:]],
                    )

            # ============== phase B: fused SpMM + epilogues ================
            gtile = {}
            n_issued = 0

            def emit_gather(b, q):
                nonlocal n_issued
                n = CBC[b][q]
                gt = gbuf.tile([P, max_grp, W2], fp8, tag="gbuf",
                               name=f"gt_{b}_{q}")
                if n > 0:
                    sc = strm_base[q][b]
                    nidx = n * P
                    nc.gpsimd.dma_gather(
                        gt[:, 0:n, :], cc_out[q][:],
                        idx_t[q][:, sc * 8:(sc + n) * 8],
                        num_idxs=nidx, num_idxs_reg=nidx, elem_size=W2,
                        single_packet=False,
                        queue_num=n_issued % 4,
                    )
                    n_issued += 1
                gtile[(b, q)] = gt

            def selu_into(dst_ap, psum_ap):
                """dst = selu(psum), mostly on ScalarE."""
                e2 = tmpp.tile([P, D], f32, tag="tmpd")
                nc.scalar.activation(e2[:], psum_ap,
                                     mybir.ActivationFunctionType.Exp,
                                     bias=ln_la[:])
                e3 = tmpp.tile([P, D], f32, tag="tmpd2")
                nc.scalar.activation(e3[:], e2[:],
                                     mybir.ActivationFunctionType.Relu,
                                     bias=la_c[:], scale=-1.0)
                rl = tmpp.tile([P, D], f32, tag="tmpd3")
                nc.scalar.activation(rl[:], psum_ap,
                                     mybir.ActivationFunctionType.Relu,
                                     scale=SELU_L)
                nc.vector.tensor_tensor(dst_ap, rl[:], e3[:],
                                        mybir.AluOpType.subtract)

            def emit_range_stats(rr):
                r0 = rblk[rr] * P
                r1 = min(rblk[rr + 1] * P, NSH)
                for t in range(DT):
                    nmt = smallp.tile([P, 1], f32, tag="nmt")
                    nc.vector.reduce_max(nmt[:], gcn1T[:, t, r0:r1],
                                         axis=mybir.AxisListType.X,
                                         negate=True)
                    o = rr * 2 * DT
                    nc.scalar.mul(stats_p[:, o + t:o + t + 1], nmt[:], -1.0)
                    exb = expp.tile([P, crmax], bf16, tag="expp")
                    nc.scalar.activation(
                        exb[:, 0:r1 - r0], gcn1T[:, t, r0:r1],
                        mybir.ActivationFunctionType.Exp, bias=nmt[:],
                        accum_out=stats_p[:, o + DT + t:o + DT + t + 1])

            with tc.tile_pool(name="partp", bufs=1) as partp:
                # partial accumulator slab: [A@h1|A@h2] per block, bf16
                partial = partp.tile([P, NB, W2], bf16, tag="partial")
                rr_next = 0
                for q in range(NCK):
                    last = q == NCK - 1
                    for b in range(NB):
                        n = CBC[b][q]
                        oht = ohp.tile([P, max_grp, P], fp8, tag="oh")
                        if n > 0:
                            o0 = ohbase[b] + ohsub[b][q]
                            nc.sync.dma_start(oht[:, 0:n, :],
                                              oh[:, o0:o0 + n, :])
                        if (b, q) not in gtile:
                            emit_gather(b, q)
                        pt = pa.tile([P, W2], f32, space="PSUM", tag="pa")
                        nmm = 0
                        tot = n + (1 if q > 0 else 0)
                        if q > 0:
                            nc.tensor.matmul(pt[:], lhsT=identb[:],
                                             rhs=partial[:, b, :],
                                             start=True, stop=(tot == 1))
                            nmm = 1
                        elif n == 0:
                            nc.vector.memset(pt[:], 0.0)
                        gt = gtile.pop((b, q))
                        for j in range(n):
                            nc.tensor.matmul(
                                pt[:], lhsT=oht[:, j, :], rhs=gt[:, j, :],
                                start=(nmm == 0), stop=(nmm == tot - 1))
                            nmm += 1
                        if not last:
                            nc.vector.tensor_copy(partial[:, b, :], pt[:])
                            continue

                        # epilogue (last pass only)
                        g1b = tmpp.tile([P, D], f32, tag="g1b")
                        selu_into(g1b[:], pt[:, 0:D])
                        aug = tmpp.tile([P, D], f32, tag="aug")
                        selu_into(aug[:], pt[:, D:W2])
                        nc.vector.tensor_tensor(accs[:, 0:D], accs[:, 0:D],
                                                aug[:], mybir.AluOpType.add)
                        pr = tmpp.tile([P, D], f32, tag="pr")
                        nc.vector.tensor_tensor(pr[:], aug[:], g1b[:],
                                                mybir.AluOpType.mult)
                        nc.vector.tensor_tensor(accs[:, D:W2], accs[:, D:W2],
                                                pr[:], mybir.AluOpType.add)
                        for t in range(DT):
                            ptr = pb.tile([P, P], f32, space="PSUM", tag="pb")
                            nc.tensor.transpose(ptr[:],
                                                g1b[:, t * P:(t + 1) * P],
                                                ident[:])
                            nc.vector.tensor_copy(
                                gcn1T[:, t, b * P:(b + 1) * P], ptr[:])
                        pl = pb.tile([P, K], f32, space="PSUM", tag="pb")
                        for t in range(DT):
                            nc.tensor.matmul(
                                pl[:], lhsT=gcn1T[:, t, b * P:(b + 1) * P],
                                rhs=wtt_t[:, t, :],
                                start=(t == 0), stop=(t == DT - 1))
                        nmx = smallp.tile([P, 1], f32, tag="nmx")
                        nc.vector.reduce_max(nmx[:], pl[:],
                                             axis=mybir.AxisListType.X,
                                             negate=True)
                        ex = smallp.tile([P, K], f32, tag="ex")
                        sm = smallp.tile([P, 1], f32, tag="sm")
                        nc.scalar.activation(ex[:], pl[:],
                                             mybir.ActivationFunctionType.Exp,
                                             bias=nmx[:], accum_out=sm[:])
                        rc = smallp.tile([P, 1], f32, tag="rc")
                        nc.vector.reciprocal(rc[:], sm[:])
                        sb = stagep.tile([P, K], f32, tag="sstage")
                        nc.scalar.mul(sb[:], ex[:], rc[:])
                        nc.sync.dma_start(s_out_d[b * P:(b + 1) * P, :],
                                          sb[:])

                        if b + 1 == rblk[rr_next + 1]:
                            emit_range_stats(rr_next)
                            rr_next += 1

            # ================= finale =================
            pv = pvp.tile([P, VLEN], f32, space="PSUM", tag="pv")
            nc.tensor.matmul(pv[0:1, :], lhsT=ones[:], rhs=accs[:],
                             start=True, stop=True)
            sv = svp.tile([1, VLEN], f32, tag="sv")
            nc.vector.tensor_copy(sv[:], pv[0:1, :])
            nc.sync.dma_start(stats_v_d[:], sv[:])
            nc.sync.dma_start(stats_p_d[:], stats_p[:])

    nc.compile()
    return nc


# --------------------------------------------------------------------------
# host-side combine of per-core partials
# --------------------------------------------------------------------------

def combine(results, cfg, graph_row, graph_col, gn_vals):
    N, D, K = cfg["N"], cfg["D"], cfg["K"]
    NSH = N // NC
    DT = D // P
    E = float(graph_row.shape[0])

    m, s = [], []
    colsum_aug = np.zeros(D)
    dot = 0.0
    S_full = np.zeros((N, K))
    for c in range(NC):
        sp = np.asarray(results[c]["stats_p"], dtype=np.float64)
        svv = np.asarray(results[c]["stats_v"], dtype=np.float64).reshape(-1)
        for rr in range(R):
            o = rr * 2 * DT
            m.append(np.concatenate([sp[:, o + t] for t in range(DT)]))
            s.append(np.concatenate([sp[:, o + DT + t] for t in range(DT)]))
        colsum_aug += svv[0:D]
        dot += svv[D:2 * D].sum()
        S_full[c * NSH:(c + 1) * NSH] = \
            np.asarray(results[c]["s_out"], dtype=np.float64)[:NSH]
    m = np.stack(m)
    s = np.stack(s)
    M = m.max(axis=0)
    Sg = (np.exp(m - M) * s).sum(axis=0)
    logZ = M + np.log(Sg)

    row = np.asarray(graph_row).astype(np.int64)
    col = np.asarray(graph_col).astype(np.int64)
    deg = np.bincount(col, minlength=N).astype(np.float64)

    trace_gp = np.einsum('ek,ek->', S_full[row], S_full[col])
    nl = S_full.T @ deg
    clsz = S_full.sum(axis=0)

    spectral = -(trace_gp - (nl ** 2).sum() / (2.0 * E)) / (2.0 * E)
    cluster = (np.linalg.norm(clsz) / N * math.sqrt(K) - 1.0) * CLUSTER_REG
    con = -(dot - (logZ * colsum_aug).sum()) / D
    return spectral + cluster + CON_REG * con


# --------------------------------------------------------------------------
# entry point
# --------------------------------------------------------------------------

_BUILD_CACHE = {}


def kernel(features, aug_features, graph_row, graph_col, graph_vals, gn_vals,
           lbl, dense_graph, W1, b1, Wt, bt, _cfg=None, _trace=False):
    cfg = _cfg or FULL
    in_maps, meta = prep(features, aug_features, graph_row, graph_col,
                         gn_vals, W1, Wt, cfg)
    key = tuple(sorted((k, str(v)) for k, v in meta.items()))
    if key not in _BUILD_CACHE:
        _BUILD_CACHE[key] = build(meta)
    nc = _BUILD_CACHE[key]
    res = bass_utils.run_bass_kernel_spmd(nc, in_maps, core_ids=list(range(NC)),
                                          trace=_trace)
    loss = combine(res.results, cfg, graph_row, graph_col, gn_vals)
    out = np.array(loss, dtype=np.float32)
    if _trace:
        return out, res
    return out


# revision 13
# speedup vs baseline: 1.3539x; 1.3539x over previous
"""Trainium2 8-core kernel for nn_CAT_81269371175150 (GNN message passing).

Math (see reference):
  gcn(x)   = selu(A_gn @ (x @ W1^T))            for features and aug_features
  S        = softmax_K(gcn1 @ Wt^T)
  loss     = spectral(S, A) + cluster(S) + 0.5 * con(gcn1, gcn2)

Strategy (v3):
  * Nodes sharded row-wise across 8 cores; edge list bucketed by
    (dest block, src chunk) with 4 src chunks, sorted, padded to the
    per-(b,c) max chunk count over cores so all cores run one SPMD program.
  * h = [h1|h2] rows stored fp8e4 (512B/row).  Phase A computes h per src
    chunk and fires that chunk's AllGather immediately -> the 4 AllGathers
    pipeline with phase A's tail and phase B's head.
  * Phase B: per (dest block, src chunk) dma_gather of fp8 rows on round-
    robin SWDGE queues + one fp8 matmul per 128-edge chunk (one-hot lhsT
    with gn folded in) accumulating [A@h1 | A@h2] in PSUM.
  * selu on ScalarE; S = softmax_K written out; log-softmax stats computed
    per node-range (4 ranges) to hide the tail; con-loss partials on-chip.
  * Host finishes tiny reductions (trace(S^T A S), nl, cluster sizes,
    cross-core/range log-softmax merge, final scalar).
"""

import math
import numpy as np
import ml_dtypes

import concourse.bacc as bacc
import concourse.mybir as mybir
import concourse.tile as tile
from concourse import bass_utils
from concourse.masks import make_identity

P = 128
NC = 8
R = 4                     # log-softmax stat ranges

FULL = dict(N=50000, F=500, D=256, K=16)

SELU_L = 1.0507009873554805
SELU_A = 1.6732632423543772
SELU_LA = SELU_L * SELU_A
LN_SELU_LA = math.log(SELU_LA)

CLUSTER_REG = 1.0
CON_REG = 0.5

bf16 = mybir.dt.bfloat16
fp8 = mybir.dt.float8e4
f32 = mybir.dt.float32
i16 = mybir.dt.int16
np_fp8 = ml_dtypes.float8_e4m3

# src chunks in blocks (sum == NB); first small so AllGather 0 fires early
CHUNK_BLOCKS = (7, 14, 14, 14)


def cdiv(a, b):
    return -(-a // b)


# --------------------------------------------------------------------------
# host-side preprocessing
# --------------------------------------------------------------------------

def prep(features, aug_features, graph_row, graph_col, gn_vals, W1, Wt, cfg):
    N, F, D, K = cfg["N"], cfg["F"], cfg["D"], cfg["K"]
    NSH = N // NC
    NB = cdiv(NSH, P)
    NCK = len(CHUNK_BLOCKS)
    cblk = np.asarray(CHUNK_BLOCKS)
    assert cblk.sum() == NB
    cb0 = np.concatenate([[0], np.cumsum(cblk)[:-1]])      # chunk start block
    crow0 = cb0 * P                                        # chunk start row
    crows = np.minimum((cb0 + cblk) * P, NSH) - crow0      # chunk rows

    row = np.asarray(graph_row).astype(np.int64)
    col = np.asarray(graph_col).astype(np.int64)
    gn = np.asarray(gn_vals).astype(np.float64)

    core = row // NSH
    per_core = []
    cnts = np.zeros((NC, NB, NCK), dtype=np.int64)
    for c in range(NC):
        m = core == c
        r = row[m] - c * NSH
        cl = col[m]
        g = gn[m]
        b = r // P
        sr = cl % NSH
        sc = np.searchsorted(crow0, sr, side="right") - 1
        loc = (cl // NSH) * crows[sc] + (sr - crow0[sc])
        order = np.lexsort((cl, sc, b))
        r, cl, g, b, sc, loc = (x[order] for x in (r, cl, g, b, sc, loc))
        key = b * NCK + sc
        cnt = np.bincount(key, minlength=NB * NCK).reshape(NB, NCK)
        cnts[c] = cnt
        per_core.append((r, g, b, sc, loc, key))

    CBC = np.ceil(cnts.max(axis=0) / P).astype(np.int64)   # [NB, NCK]
    nch_b = CBC.sum(axis=1)                                # chunks per block
    NCHT = int(nch_b.sum())
    ohbase = np.concatenate([[0], np.cumsum(nch_b)[:-1]])
    ohsub = np.cumsum(CBC, axis=1) - CBC                   # col offset of (b,c)
    strm_base = np.zeros((NCK, NB), dtype=np.int64)
    for q in range(NCK):
        strm_base[q] = np.concatenate([[0], np.cumsum(CBC[:, q])[:-1]])
    Lc = [int(CBC[:, q].sum()) * P for q in range(NCK)]

    X = np.asarray(features)[0]
    Xa = np.asarray(aug_features)[0]
    XT = np.ascontiguousarray(X.T).astype(ml_dtypes.bfloat16)    # [F, N]
    XTa = np.ascontiguousarray(Xa.T).astype(ml_dtypes.bfloat16)
    W1T = np.ascontiguousarray(np.asarray(W1).T).astype(ml_dtypes.bfloat16)
    WtT = np.ascontiguousarray(np.asarray(Wt).T).astype(ml_dtypes.bfloat16)

    def wrap_idx(a):
        # [L] -> [128, L/16]: element i at [i%16, i//16], replicated x8
        L = a.shape[0]
        w = a.reshape(L // 16, 16).T
        return np.ascontiguousarray(np.tile(w, (8, 1)))

    in_maps = []
    for c in range(NC):
        r, g, b, sc, loc, key = per_core[c]
        cnt = cnts[c]
        run_start = np.zeros(NB * NCK, dtype=np.int64)
        flat = cnt.reshape(-1)
        run_start[1:] = np.cumsum(flat)[:-1]
        rank = np.arange(len(r)) - run_start[key]
        lane = rank % P
        j = rank // P

        idx_streams = []
        for q in range(NCK):
            arr = np.zeros(Lc[q], dtype=np.int16)
            m = sc == q
            off = (strm_base[q][b[m]] + j[m]) * P + lane[m]
            arr[off] = loc[m].astype(np.int16)
            idx_streams.append(wrap_idx(arr))

        oh = np.zeros((P, NCHT, P), dtype=np_fp8)
        ohcol = ohbase[b] + ohsub[b, sc] + j
        dest = r - b * P
        oh[lane, ohcol, dest] = g.astype(np_fp8)

        in_maps.append({
            "xt": np.ascontiguousarray(XT[:, c * NSH:(c + 1) * NSH]),
            "xta": np.ascontiguousarray(XTa[:, c * NSH:(c + 1) * NSH]),
            "w1t": W1T,
            "wtt": WtT,
            "oh": oh,
            **{f"idx{q}": idx_streams[q] for q in range(NCK)},
        })

    meta = dict(
        N=N, F=F, D=D, K=K, NSH=NSH, NB=NB, DT=D // P, FT=cdiv(F, P),
        NCK=NCK, cblk=tuple(cblk.tolist()), cb0=tuple(cb0.tolist()),
        crow0=tuple(crow0.tolist()), crows=tuple(crows.tolist()),
        CBC=tuple(map(tuple, CBC.tolist())), NCHT=NCHT,
        ohbase=tuple(ohbase.tolist()),
        ohsub=tuple(map(tuple, ohsub.tolist())),
        strm_base=tuple(map(tuple, strm_base.tolist())),
        Lc=tuple(Lc),
    )
    return in_maps, meta


# --------------------------------------------------------------------------
# device program
# --------------------------------------------------------------------------

def build(meta, debug=False):
    N, F, D, K = meta["N"], meta["F"], meta["D"], meta["K"]
    NSH, NB, DT, FT = meta["NSH"], meta["NB"], meta["DT"], meta["FT"]
    NCK = meta["NCK"]
    cblk, cb0 = meta["cblk"], meta["cb0"]
    crow0, crows = meta["crow0"], meta["crows"]
    CBC = meta["CBC"]
    NCHT = meta["NCHT"]
    ohbase, ohsub = meta["ohbase"], meta["ohsub"]
    strm_base = meta["strm_base"]
    Lc = meta["Lc"]
    W2 = 2 * D
    VLEN = 2 * D

    # log-softmax ranges: block boundaries
    rblk = [0, 13, 26, 39, NB]
    assert len(rblk) == R + 1
    BPRE0 = 36                # blocks >= BPRE0 use SBUF-preloaded one-hots

    nc = bacc.Bacc("TRN2", target_bir_lowering=False, debug=debug,
                   num_devices=NC, num_swdge_queues=4)

    xt = nc.dram_tensor("xt", [F, NSH], bf16, kind="ExternalInput")
    xta = nc.dram_tensor("xta", [F, NSH], bf16, kind="ExternalInput")
    w1t = nc.dram_tensor("w1t", [F, D], bf16, kind="ExternalInput")
    wtt = nc.dram_tensor("wtt", [D, K], bf16, kind="ExternalInput")
    oh = nc.dram_tensor("oh", [P, NCHT, P], fp8, kind="ExternalInput")
    idx_d = [nc.dram_tensor(f"idx{q}", [P, Lc[q] // 16], i16,
                            kind="ExternalInput")
             for q in range(NCK)]

    stats_p_d = nc.dram_tensor("stats_p", [P, R * 2 * DT], f32,
                               kind="ExternalOutput")
    stats_v_d = nc.dram_tensor("stats_v", [1, VLEN], f32,
                               kind="ExternalOutput")
    s_out_d = nc.dram_tensor("s_out", [NB * P, K], f32, kind="ExternalOutput")

    max_nch_b = max(sum(CBC[b]) for b in range(NB))
    max_grp = max(CBC[b][q] for b in range(NB) for q in range(NCK))
    crmax = max(crows)

    with tile.TileContext(nc) as tc:
        with (
            tc.tile_pool(name="xp", bufs=4) as xp,
            tc.tile_pool(name="gbuf", bufs=12) as gbuf,
            tc.tile_pool(name="ohp", bufs=3) as ohp,
            tc.tile_pool(name="persist", bufs=1) as persist,
            tc.tile_pool(name="stage", bufs=4) as stagep,
            tc.tile_pool(name="expp", bufs=1) as expp,
            tc.tile_pool(name="tmp", bufs=4) as tmpp,
            tc.tile_pool(name="small", bufs=4) as smallp,
            tc.tile_pool(name="svp", bufs=1) as svp,
            tc.tile_pool(name="pa", bufs=2, space="PSUM") as pa,
            tc.tile_pool(name="pb", bufs=4, space="PSUM") as pb,
            tc.tile_pool(name="pv", bufs=1, space="PSUM") as pvp,
            tc.tile_pool(name="dram", bufs=1, space="DRAM") as dramp,
        ):
            # ---- constants / resident tensors
            ident = persist.tile([P, P], f32)
            make_identity(nc, ident[:])
            w1t_t = persist.tile([P, FT, D], bf16)
            for t in range(FT):
                fr = min(P, F - t * P)
                nc.sync.dma_start(w1t_t[:fr, t, :], w1t[t * P:t * P + fr, :])
            wtt_t = persist.tile([P, DT, K], bf16)
            for t in range(DT):
                nc.sync.dma_start(wtt_t[:, t, :], wtt[t * P:(t + 1) * P, :])
            idx_t = []
            for q in range(NCK):
                it = persist.tile([P, Lc[q] // 16], i16, tag=f"idx{q}")
                nc.sync.dma_start(it[:], idx_d[q][:])
                idx_t.append(it)

            ln_la = persist.tile([P, 1], f32, tag="lnla")
            nc.vector.memset(ln_la[:], LN_SELU_LA)
            la_c = persist.tile([P, 1], f32, tag="lac")
            nc.vector.memset(la_c[:], SELU_LA)
            ones = persist.tile([P, 1], f32, tag="ones")
            nc.vector.memset(ones[:], 1.0)

            gcn1T = persist.tile([P, DT, NB * P], bf16, tag="gcn1T")
            accs = persist.tile([P, VLEN], f32, tag="accs")
            nc.vector.memset(accs[:], 0.0)
            stats_p = persist.tile([P, R * 2 * DT], f32, tag="statsp")
            ohpre = persist.tile([P, NCHT - ohbase[BPRE0], P], fp8,
                                 tag="ohpre")

            cc_in = [dramp.tile([crows[q], W2], fp8, name=f"cc_in{q}")
                     for q in range(NCK)]
            cc_out = [dramp.tile([NC * crows[q], W2], fp8,
                                 addr_space="Shared", name=f"cc_out{q}")
                      for q in range(NCK)]

            # ============== phase A + AllGathers (chunk-pipelined) =========
            for q in range(NCK):
                xts = []
                for which, src in enumerate((xt, xta)):
                    xx = xp.tile([P, FT, crmax], bf16, tag="xp",
                                 name=f"x_{which}_{q}")
                    for t in range(FT):
                        fr = min(P, F - t * P)
                        nc.sync.dma_start(
                            xx[:fr, t, 0:crows[q]],
                            src[t * P:t * P + fr,
                                crow0[q]:crow0[q] + crows[q]])
                    xts.append(xx)
                for b in range(cb0[q], cb0[q] + cblk[q]):
                    boff = b * P - crow0[q]
                    rows = min(P, NSH - b * P)
                    st = stagep.tile([P, W2], fp8, tag="stage")
                    for which in range(2):
                        pt = pb.tile([P, D], f32, space="PSUM", tag="pb")
                        for t in range(FT):
                            fr = min(P, F - t * P)
                            nc.tensor.matmul(
                                pt[:rows, :],
                                lhsT=xts[which][:fr, t, boff:boff + rows],
                                rhs=w1t_t[:fr, t, :],
                                start=(t == 0), stop=(t == FT - 1),
                            )
                        nc.vector.tensor_copy(
                            st[:rows, which * D:(which + 1) * D],
                            pt[:rows, :])
                    nc.sync.dma_start(cc_in[q][boff:boff + rows, :],
                                      st[:rows, :])
                nc.gpsimd.collective_compute(
                    "AllGather", mybir.AluOpType.bypass,
                    replica_groups=[list(range(NC))],
                    ins=[cc_in[q][:]], outs=[cc_out[q][:]],
                )

            # preload the tail blocks' one-hots while the AllGather chain
            # runs (the DMA engines are mostly idle in that window)
            nc.sync.dma_start(ohpre[:], oh[:, ohbase[BPRE0]:NCHT, :])

            # ============== phase B: fused SpMM + epilogues ================
            gtile = {}
            n_issued = 0

            def emit_gather(b, q):
                nonlocal n_issued
                n = CBC[b][q]
                gt = gbuf.tile([P, max_grp, W2], fp8, tag="gbuf",
                               name=f"gt_{b}_{q}")
                if n > 0:
                    sc = strm_base[q][b]
                    nidx = n * P
                    nc.gpsimd.dma_gather(
                        gt[:, 0:n, :], cc_out[q][:],
                        idx_t[q][:, sc * 8:(sc + n) * 8],
                        num_idxs=nidx, num_idxs_reg=nidx, elem_size=W2,
                        single_packet=False,
                        queue_num=n_issued % 4,
                    )
                    n_issued += 1
                gtile[(b, q)] = gt

            # priming: early blocks' gathers right after their AllGather
            PRIME = 3
            for q in range(NCK):
                for b in range(min(PRIME, NB)):
                    emit_gather(b, q)

            def selu_into(dst_ap, psum_ap):
                """dst = selu(psum), mostly on ScalarE."""
                e2 = tmpp.tile([P, D], f32, tag="tmpd")
                nc.scalar.activation(e2[:], psum_ap,
                                     mybir.ActivationFunctionType.Exp,
                                     bias=ln_la[:])
                e3 = tmpp.tile([P, D], f32, tag="tmpd2")
                nc.scalar.activation(e3[:], e2[:],
                                     mybir.ActivationFunctionType.Relu,
                                     bias=la_c[:], scale=-1.0)
                rl = tmpp.tile([P, D], f32, tag="tmpd3")
                nc.scalar.activation(rl[:], psum_ap,
                                     mybir.ActivationFunctionType.Relu,
                                     scale=SELU_L)
                nc.vector.tensor_tensor(dst_ap, rl[:], e3[:],
                                        mybir.AluOpType.subtract)

            def emit_range_stats(rr):
                r0 = rblk[rr] * P
                r1 = min(rblk[rr + 1] * P, NSH)
                for t in range(DT):
                    nmt = smallp.tile([P, 1], f32, tag="nmt")
                    nc.vector.reduce_max(nmt[:], gcn1T[:, t, r0:r1],
                                         axis=mybir.AxisListType.X,
                                         negate=True)
                    o = rr * 2 * DT
                    nc.scalar.mul(stats_p[:, o + t:o + t + 1], nmt[:], -1.0)
                    exb = expp.tile([P, crmax], bf16, tag="expp")
                    nc.scalar.activation(
                        exb[:, 0:r1 - r0], gcn1T[:, t, r0:r1],
                        mybir.ActivationFunctionType.Exp, bias=nmt[:],
                        accum_out=stats_p[:, o + DT + t:o + DT + t + 1])

            rr_next = 0
            for b in range(NB):
                rows = min(P, NSH - b * P)
                nch = sum(CBC[b])
                if b >= BPRE0:
                    oht = ohpre
                    obase = ohbase[b] - ohbase[BPRE0]
                else:
                    oht = ohp.tile([P, max_nch_b, P], fp8, tag="oh")
                    obase = 0
                    nc.sync.dma_start(oht[:, 0:nch, :],
                                      oh[:, ohbase[b]:ohbase[b] + nch, :])
                for q in range(NCK):
                    if (b, q) not in gtile:
                        emit_gather(b, q)
                pt = pa.tile([P, W2], f32, space="PSUM", tag="pa")
                tot = nch
                nmm = 0
                if tot == 0:
                    nc.vector.memset(pt[:], 0.0)
                for q in range(NCK):
                    gt = gtile.pop((b, q))
                    for j in range(CBC[b][q]):
                        nc.tensor.matmul(
                            pt[:], lhsT=oht[:, obase + ohsub[b][q] + j, :],
                            rhs=gt[:, j, :],
                            start=(nmm == 0), stop=(nmm == tot - 1))
                        nmm += 1

                # epilogue
                g1b = tmpp.tile([P, D], f32, tag="g1b")
                selu_into(g1b[:], pt[:, 0:D])
                aug = tmpp.tile([P, D], f32, tag="aug")
                selu_into(aug[:], pt[:, D:W2])
                nc.vector.tensor_tensor(accs[:, 0:D], accs[:, 0:D], aug[:],
                                        mybir.AluOpType.add)
                pr = tmpp.tile([P, D], f32, tag="pr")
                nc.vector.tensor_tensor(pr[:], aug[:], g1b[:],
                                        mybir.AluOpType.mult)
                nc.vector.tensor_tensor(accs[:, D:W2], accs[:, D:W2], pr[:],
                                        mybir.AluOpType.add)
                for t in range(DT):
                    ptr = pb.tile([P, P], f32, space="PSUM", tag="pb")
                    nc.tensor.transpose(ptr[:], g1b[:, t * P:(t + 1) * P],
                                        ident[:])
                    nc.vector.tensor_copy(gcn1T[:, t, b * P:(b + 1) * P],
                                          ptr[:])
                pl = pb.tile([P, K], f32, space="PSUM", tag="pb")
                for t in range(DT):
                    nc.tensor.matmul(pl[:],
                                     lhsT=gcn1T[:, t, b * P:(b + 1) * P],
                                     rhs=wtt_t[:, t, :],
                                     start=(t == 0), stop=(t == DT - 1))
                nmx = smallp.tile([P, 1], f32, tag="nmx")
                nc.vector.reduce_max(nmx[:], pl[:], axis=mybir.AxisListType.X,
                                     negate=True)
                ex = smallp.tile([P, K], f32, tag="ex")
                sm = smallp.tile([P, 1], f32, tag="sm")
                nc.scalar.activation(ex[:], pl[:],
                                     mybir.ActivationFunctionType.Exp,
                                     bias=nmx[:], accum_out=sm[:])
                rc = smallp.tile([P, 1], f32, tag="rc")
                nc.vector.reciprocal(rc[:], sm[:])
                sb = stagep.tile([P, K], f32, tag="sstage")
                nc.scalar.mul(sb[:], ex[:], rc[:])
                nc.sync.dma_start(s_out_d[b * P:(b + 1) * P, :], sb[:])

                if b + 1 == rblk[rr_next + 1]:
                    emit_range_stats(rr_next)
                    rr_next += 1

            # ================= finale =================
            pv = pvp.tile([P, VLEN], f32, space="PSUM", tag="pv")
            nc.tensor.matmul(pv[0:1, :], lhsT=ones[:], rhs=accs[:],
                             start=True, stop=True)
            sv = svp.tile([1, VLEN], f32, tag="sv")
            nc.vector.tensor_copy(sv[:], pv[0:1, :])
            nc.sync.dma_start(stats_v_d[:], sv[:])
            nc.sync.dma_start(stats_p_d[:], stats_p[:])

    nc.compile()
    return nc


# --------------------------------------------------------------------------
# host-side combine of per-core partials
# --------------------------------------------------------------------------

def combine(results, cfg, graph_row, graph_col, gn_vals):
    N, D, K = cfg["N"], cfg["D"], cfg["K"]
    NSH = N // NC
    DT = D // P
    E = float(graph_row.shape[0])

    m, s = [], []
    colsum_aug = np.zeros(D)
    dot = 0.0
    S_full = np.zeros((N, K))
    for c in range(NC):
        sp = np.asarray(results[c]["stats_p"], dtype=np.float64)
        svv = np.asarray(results[c]["stats_v"], dtype=np.float64).reshape(-1)
        for rr in range(R):
            o = rr * 2 * DT
            m.append(np.concatenate([sp[:, o + t] for t in range(DT)]))
            s.append(np.concatenate([sp[:, o + DT + t] for t in range(DT)]))
        colsum_aug += svv[0:D]
        dot += svv[D:2 * D].sum()
        S_full[c * NSH:(c + 1) * NSH] = \
            np.asarray(results[c]["s_out"], dtype=np.float64)[:NSH]
    m = np.stack(m)
    s = np.stack(s)
    M = m.max(axis=0)
    Sg = (np.exp(m - M) * s).sum(axis=0)
    logZ = M + np.log(Sg)

    row = np.asarray(graph_row).astype(np.int64)
    col = np.asarray(graph_col).astype(np.int64)
    deg = np.bincount(col, minlength=N).astype(np.float64)

    trace_gp = np.einsum('ek,ek->', S_full[row], S_full[col])
    nl = S_full.T @ deg
    clsz = S_full.sum(axis=0)

    spectral = -(trace_gp - (nl ** 2).sum() / (2.0 * E)) / (2.0 * E)
    cluster = (np.linalg.norm(clsz) / N * math.sqrt(K) - 1.0) * CLUSTER_REG
    con = -(dot - (logZ * colsum_aug).sum()) / D
    return spectral + cluster + CON_REG * con


# --------------------------------------------------------------------------
# entry point
# --------------------------------------------------------------------------

_BUILD_CACHE = {}


def kernel(features, aug_features, graph_row, graph_col, graph_vals, gn_vals,
           lbl, dense_graph, W1, b1, Wt, bt, _cfg=None, _trace=False):
    cfg = _cfg or FULL
    in_maps, meta = prep(features, aug_features, graph_row, graph_col,
                         gn_vals, W1, Wt, cfg)
    key = tuple(sorted((k, str(v)) for k, v in meta.items()))
    if key not in _BUILD_CACHE:
        _BUILD_CACHE[key] = build(meta)
    nc = _BUILD_CACHE[key]
    res = bass_utils.run_bass_kernel_spmd(nc, in_maps, core_ids=list(range(NC)),
                                          trace=_trace)
    loss = combine(res.results, cfg, graph_row, graph_col, gn_vals)
    out = np.array(loss, dtype=np.float32)
    if _trace:
        return out, res
    return out
